# revision 8
# baseline (speedup 1.0000x reference)
"""Trainium2 Bass kernel for nn_AggregateClusteredSum.

Data-parallel over the batch axis: 32 rows / 8 NeuronCores = 4 rows per core.
Per row, segment sums of hs over 64 clusters are computed as accumulating
matmuls with on-device one-hot matrices (built 8 chunks at a time by a single
DVE is_equal over zero-stride broadcast views of cs and an iota constant).
The one-hot is the stationary operand (64-wide weight loads), giving a
cluster-major [64, 128] accumulator that is transposed once per row on the
PE. The 6-layer PReLU MLP runs feature-major over all 4*129 tokens at once
(natural weight layout stationary, bias+PReLU fused into the ACT eviction),
followed by the leave-one-out aggregation on DVE and a final PE transpose
per row.

Host-side work is limited to index metadata: the exists mask (bincount>0 of
cs), the Ks-based tail reassignment and G_mask (pure functions of cs), and
input slicing/gather.
"""
import os
import sys

for _p in ("/opt/trn_rl_repo", "/root/.axon_site/_ro/trn_rl_repo"):
    if os.path.isdir(_p) and _p not in sys.path:
        sys.path.insert(0, _p)

import numpy as np
from contextlib import ExitStack

import concourse.bass as bass
import concourse.tile as tile
from concourse import bacc, mybir
from concourse.bass_utils import run_bass_kernel_spmd

F32 = mybir.dt.float32
F16 = mybir.dt.float16
BF16 = mybir.dt.bfloat16

N_CORES = 8
K = 64                      # clusters
H = 128                     # hidden dim of hs
G_DIM = 128                 # output dim
HID = 256                   # MLP hidden
P = 128                     # partitions
NB = 8                      # one-hot chunks built per DVE op
NSPLIT = 1                  # hs DMA split per row

_PROGRAM_CACHE = {}
LAST_RESULT = None          # BassKernelResults of the most recent run (for profiling)
TRACE = False


def _build_program(rows_per_core, n, alphas):
    """Build the per-core Bass program. Same program for all cores (SPMD)."""
    nch = n // P            # full 128-row chunks per batch row
    rem = n - nch * P       # remainder rows (0 for n=4096)
    ntok = 2 * K + 1        # 129 tokens per row
    T = rows_per_core * ntok  # total tokens per core (516)
    # token free-dim chunks for the MLP (PSUM bank limit: 512 f32)
    nt = (T + 511) // 512
    base = T // nt
    tchunks = []
    t0 = 0
    for i in range(nt):
        tw = base + (1 if i < T - base * nt else 0)
        tchunks.append((t0, tw))
        t0 += tw
    assert t0 == T

    cs_cols = nch + (1 if rem > 0 else 0)
    nc = bacc.Bacc()

    hs_in = nc.declare_dram_parameter("hs4", [rows_per_core, n + 1, H], F32, isOutput=False)
    cs_in = nc.declare_dram_parameter("cs4", [rows_per_core, P, max(cs_cols, 1)], F32, isOutput=False)
    em_in = nc.declare_dram_parameter("em4", [rows_per_core, P, K], F32, isOutput=False)
    iota_in = nc.declare_dram_parameter("iota", [P, K], F32, isOutput=False)
    ident_in = nc.declare_dram_parameter("ident", [P, P], F32, isOutput=False)
    id16_in = nc.declare_dram_parameter("id16", [K + 1, K + 1], F16, isOutput=False)
    w1_in = nc.declare_dram_parameter("w1", [H, HID], F32, isOutput=False)
    w2_in = nc.declare_dram_parameter("w2", [2, P, HID], F32, isOutput=False)
    w3_in = nc.declare_dram_parameter("w3", [2, P, HID], F32, isOutput=False)
    w4_in = nc.declare_dram_parameter("w4", [2, P, HID], F32, isOutput=False)
    w5_in = nc.declare_dram_parameter("w5", [2, P, HID], F32, isOutput=False)
    w6_in = nc.declare_dram_parameter("w6", [2, P, G_DIM], F32, isOutput=False)
    b1_in = nc.declare_dram_parameter("b1", [2, P, 1], F32, isOutput=False)
    b2_in = nc.declare_dram_parameter("b2", [2, P, 1], F32, isOutput=False)
    b3_in = nc.declare_dram_parameter("b3", [2, P, 1], F32, isOutput=False)
    b4_in = nc.declare_dram_parameter("b4", [2, P, 1], F32, isOutput=False)
    b5_in = nc.declare_dram_parameter("b5", [2, P, 1], F32, isOutput=False)
    b6_in = nc.declare_dram_parameter("b6", [P, 1], F32, isOutput=False)
    g_out = nc.declare_dram_parameter("g4", [rows_per_core, K + 1, G_DIM], F32, isOutput=True)

    a1, a2, a3, a4, a5 = [float(a) for a in alphas]
    Act = mybir.ActivationFunctionType
    Alu = mybir.AluOpType

    with tile.TileContext(nc) as tc, ExitStack() as ctx:
        consts = ctx.enter_context(tc.tile_pool(name="consts", bufs=1))
        wpool = ctx.enter_context(tc.tile_pool(name="wpool", bufs=1))
        hspool = ctx.enter_context(tc.tile_pool(name="hspool", bufs=2 * NSPLIT))
        small = ctx.enter_context(tc.tile_pool(name="small", bufs=2))
        ohpool = ctx.enter_context(tc.tile_pool(name="ohpool", bufs=4))
        xpool = ctx.enter_context(tc.tile_pool(name="xpool", bufs=1))
        loopool = ctx.enter_context(tc.tile_pool(name="loopool", bufs=2))
        pseg = ctx.enter_context(tc.tile_pool(name="pseg", bufs=2, space="PSUM"))
        ptp = ctx.enter_context(tc.tile_pool(name="ptp", bufs=1, space="PSUM"))
        pmlp = ctx.enter_context(tc.tile_pool(name="pmlp", bufs=4, space="PSUM"))

        # constants
        iota_sb = consts.tile([P, K], F32)
        nc.sync.dma_start(out=iota_sb[:], in_=iota_in[:])
        ident_sb = consts.tile([P, P], F32)
        nc.sync.dma_start(out=ident_sb[:], in_=ident_in[:])
        id16_sb = consts.tile([K + 1, K + 1], F16)
        nc.sync.dma_start(out=id16_sb[:], in_=id16_in[:])

        # weights: raw f32 on the HWDGE queue (keeps SWDGE free for hs),
        # converted to fp16 on ACT
        w1_f = wpool.tile([P, HID], F32, tag="w1f")
        nc.sync.dma_start(out=w1_f[:], in_=w1_in[:])
        w1_sb = wpool.tile([P, HID], F16, tag="w1")
        nc.scalar.copy(w1_sb[:], w1_f[:])
        w_sb = {}
        for li, w_in in ((2, w2_in), (3, w3_in), (4, w4_in), (5, w5_in), (6, w6_in)):
            for ci in range(2):
                wf = wpool.tile([P, HID if li < 6 else G_DIM], F32,
                                tag=f"w{li}_{ci}f", name=f"w{li}_{ci}f")
                nc.sync.dma_start(out=wf[:], in_=w_in[ci])
                t = wpool.tile([P, HID if li < 6 else G_DIM], F16,
                               tag=f"w{li}_{ci}", name=f"w{li}_{ci}")
                nc.scalar.copy(t[:], wf[:])
                w_sb[(li, ci)] = t
        b_sb = {}
        for li, b_in in ((1, b1_in), (2, b2_in), (3, b3_in), (4, b4_in), (5, b5_in)):
            for hi in range(2):
                t = wpool.tile([P, 1], F32, tag=f"b{li}_{hi}", name=f"b{li}_{hi}")
                nc.sync.dma_start(out=t[:], in_=b_in[hi])
                b_sb[(li, hi)] = t
        b6_sb = wpool.tile([P, 1], F32, tag="b6")
        nc.sync.dma_start(out=b6_sb[:], in_=b6_in[:])

        # X0: Hcat^T for all rows, feature-major [H, T] fp16
        x0 = xpool.tile([P, T], F16, tag="x0")

        # ---- Stage A: per-row segment sums -> X0 columns ----
        for r in range(rows_per_core):
            # hs rows [0, nch*P) in partition-block layout, split into NSPLIT
            # loads so matmuls can start before the whole row has arrived:
            # tile[p, c*H + h] = hs[r, p*nch + c, h]
            csplit = [(s * nch // NSPLIT, (s + 1) * nch // NSPLIT) for s in range(NSPLIT)]
            hs_parts = []
            for (c0, c1) in csplit:
                hp = hspool.tile([P, (c1 - c0) * H], BF16, tag=f"hs{c0}", name=f"hs_{r}_{c0}")
                # per partition p: rows p*nch + [c0, c1) -> contiguous in c,h
                src = hs_in[r, 0:nch * P, :].rearrange("(p c) h -> p c h", p=P)[:, c0:c1, :]
                nc.gpsimd.dma_start(out=hp[:].rearrange("p (c h) -> p c h", h=H), in_=src)
                hs_parts.append(hp)

            cs_t = small.tile([P, max(cs_cols, 1)], F32, tag="cs")
            nc.sync.dma_start(out=cs_t[:], in_=cs_in[r])

            hn_sb = small.tile([1, H], F32, tag="hn")
            nc.sync.dma_start(out=hn_sb[:], in_=hs_in[r, n:n + 1, :])

            psC = pseg.tile([K, P], F32, tag="psC")
            c = 0
            last = (rem == 0)
            for si, (c0, c1) in enumerate(csplit):
                hp = hs_parts[si]
                for b0 in range(c0, c1, NB):
                    bw = min(NB, c1 - b0)
                    oh = ohpool.tile([P, NB * K], BF16, tag="oh", name=f"oh_{r}_{b0}")
                    cs_b = cs_t[:, b0:b0 + bw].broadcast_to((P, bw, K))
                    io_b = iota_sb[:].unsqueeze(1).broadcast_to((P, bw, K))
                    nc.vector.tensor_tensor(
                        oh[:].rearrange("p (c k) -> p c k", k=K)[:, 0:bw, :],
                        cs_b, io_b, Alu.is_equal)
                    for cc in range(bw):
                        cg = b0 + cc          # global chunk index
                        cl = cg - c0          # chunk index within this part
                        nc.tensor.matmul(
                            psC[:], oh[:, cc * K:(cc + 1) * K],
                            hp[:, cl * H:(cl + 1) * H],
                            start=(cg == 0),
                            stop=(last and cg == nch - 1))
            if rem > 0:
                hs_r = hspool.tile([P, H], BF16, tag="hs_rem")
                nc.gpsimd.dma_start(out=hs_r[0:rem, :], in_=hs_in[r, nch * P:n, :])
                oh_r = ohpool.tile([P, K], BF16, tag="oh_rem")
                nc.vector.tensor_scalar(oh_r[0:rem, :], iota_sb[0:rem, :],
                                        cs_t[0:rem, nch:nch + 1], None, Alu.is_equal)
                nc.tensor.matmul(psC[:], oh_r[0:rem, :], hs_r[0:rem, :],
                                 start=(nch == 0), stop=True)

            # cluster-major [64+1, 128]: rows 0..63 = Hk, row 64 = hn
            cm = small.tile([K + 1, P], F16, tag="cm")
            nc.scalar.copy(cm[0:K, :], psC[:])
            nc.vector.tensor_copy(cm[K:K + 1, :], hn_sb[:])
            tps = ptp.tile([P, K + 1], F16, tag="tps")
            nc.tensor.transpose(tps[:], cm[:], id16_sb[:])

            r0 = r * ntok
            hn_col = small.tile([P, 1], F32, tag="hncol")
            nc.scalar.copy(hn_col[:], tps[:, K:K + 1])
            nc.scalar.copy(x0[:, r0:r0 + K], tps[:, 0:K])
            nc.vector.tensor_scalar(x0[:, r0 + K:r0 + 2 * K], tps[:, 0:K],
                                    hn_col[:], None, Alu.add)
            nc.vector.tensor_scalar(x0[:, r0 + 2 * K:r0 + 2 * K + 1],
                                    hn_col[:], 0.0, None, Alu.add)

        # ---- Stage B: MLP over all T tokens, feature-major ----
        x1 = [xpool.tile([P, T], F16, tag=f"x1_{h}", name=f"x1_{h}") for h in range(2)]
        for h in range(2):
            for (t0, tw) in tchunks:
                ps = pmlp.tile([P, tw], F32, tag="pmlp", name=f"ps1_{h}_{t0}")
                nc.tensor.matmul(ps[:], w1_sb[:, h * P:(h + 1) * P], x0[:, t0:t0 + tw],
                                 start=True, stop=True)
                nc.scalar.activation(x1[h][:, t0:t0 + tw], ps[:], Act.Prelu,
                                     bias=b_sb[(1, h)][:], scale=1.0, alpha=a1)
        xprev = x1
        for li, alpha in ((2, a2), (3, a3), (4, a4), (5, a5)):
            xn = [xpool.tile([P, T], F16, tag=f"x{li}_{h}", name=f"x{li}_{h}") for h in range(2)]
            for h in range(2):
                for (t0, tw) in tchunks:
                    ps = pmlp.tile([P, tw], F32, tag="pmlp", name=f"ps{li}_{h}_{t0}")
                    for ci in range(2):
                        nc.tensor.matmul(ps[:], w_sb[(li, ci)][:, h * P:(h + 1) * P],
                                         xprev[ci][:, t0:t0 + tw],
                                         start=(ci == 0), stop=(ci == 1))
                    nc.scalar.activation(xn[h][:, t0:t0 + tw], ps[:], Act.Prelu,
                                         bias=b_sb[(li, h)][:], scale=1.0, alpha=alpha)
            xprev = xn
        # L6: 256 -> 128, bias only, keep f32
        gs = xpool.tile([P, T], F32, tag="gs")
        for (t0, tw) in tchunks:
            ps = pmlp.tile([P, tw], F32, tag="pmlp", name=f"ps6_{t0}")
            for ci in range(2):
                nc.tensor.matmul(ps[:], w_sb[(6, ci)][:], xprev[ci][:, t0:t0 + tw],
                                 start=(ci == 0), stop=(ci == 1))
            nc.scalar.activation(gs[:, t0:t0 + tw], ps[:], Act.Identity,
                                 bias=b6_sb[:], scale=1.0)

        # ---- Stage C: leave-one-out per row ----
        for r in range(rows_per_core):
            r0 = r * ntok
            em_sb = loopool.tile([P, K], F32, tag="em")
            nc.sync.dma_start(out=em_sb[:], in_=em_in[r])
            scr = loopool.tile([P, K], F32, tag="scr")
            s_col = loopool.tile([P, 1], F32, tag="scol")
            # scr = gs_lo * em ; s = sum_free(scr)  (masked base sum S)
            nc.vector.scalar_tensor_tensor(scr[:], gs[:, r0:r0 + K], 1.0, em_sb[:],
                                           Alu.mult, Alu.mult, accum_out=s_col[:])
            gout = loopool.tile([P, K + 1], F32, tag="gout")
            tmp = loopool.tile([P, K], F32, tag="tmp")
            # tmp = (gs_hi + S) - gs_lo
            nc.vector.scalar_tensor_tensor(tmp[:], gs[:, r0 + K:r0 + 2 * K], s_col[:],
                                           gs[:, r0:r0 + K], Alu.add, Alu.subtract)
            nc.vector.tensor_tensor(gout[:, 0:K], tmp[:], em_sb[:], Alu.mult)
            nc.vector.tensor_scalar(gout[:, K:K + 1], gs[:, r0 + 2 * K:r0 + 2 * K + 1],
                                    s_col[:], None, Alu.add)
            # transpose [128g, 65] -> [65, 128g] and store
            tp = ptp.tile([K + 1, P], F32, tag="tp")
            nc.tensor.transpose(tp[:], gout[:], ident_sb[:])
            osb = loopool.tile([K + 1, P], F32, tag="osb")
            nc.scalar.copy(osb[:], tp[:])
            nc.sync.dma_start(out=g_out[r], in_=osb[:])

    nc.finalize()
    return nc


def kernel(**inputs):
    global LAST_RESULT
    hs = np.ascontiguousarray(np.asarray(inputs["hs"], dtype=np.float32))
    cs = np.asarray(inputs["cs"])
    n = int(np.asarray(inputs["n"]))
    B, N, _H = hs.shape
    assert _H == H and B % N_CORES == 0
    rows_per_core = B // N_CORES
    assert n >= 1
    nch = n // P

    cs_i = cs.astype(np.int64)
    cs_valid = cs_i[:, :n]                      # entries >= n are masked off

    # ---- host-side index metadata ----
    cnt = np.zeros((B, K), dtype=np.int64)
    for b in range(B):
        cnt[b] = np.bincount(cs_valid[b], minlength=K)[:K]
    exists = (cnt > 0).astype(np.float32)       # [B, K]
    Ks = cs_valid.max(axis=1)                   # [B] (cs values are >= 0)

    # ---- device inputs ----
    npad = nch * P
    rem = n - npad
    cs_cols = nch + (1 if rem > 0 else 0)
    cs_dev = np.zeros((B, P, max(cs_cols, 1)), dtype=np.float32)
    if nch > 0:
        # cs_dev[b, p, c] = cs[b, p*nch + c]  (partition-block layout)
        cs_dev[:, :, :nch] = cs_valid[:, :npad].reshape(B, P, nch)
    if rem > 0:
        cs_dev[:, :rem, nch] = cs_valid[:, npad:n]
    em = np.broadcast_to(exists[:, None, :], (B, P, K)).astype(np.float32)
    iota = np.broadcast_to(np.arange(K, dtype=np.float32), (P, K))
    ident = np.eye(P, dtype=np.float32)
    id16 = np.eye(K + 1, dtype=np.float16)

    def w_split(w):
        w = np.asarray(w, dtype=np.float32)
        return np.ascontiguousarray(w.reshape(2, P, w.shape[1]))

    def b_split(b):
        b = np.asarray(b, dtype=np.float32)
        return np.ascontiguousarray(b.reshape(2, P, 1))

    alphas = tuple(float(np.asarray(inputs[f"a{i}"])) for i in range(1, 6))
    key = (rows_per_core, n, alphas)
    if key not in _PROGRAM_CACHE:
        _PROGRAM_CACHE[key] = _build_program(rows_per_core, n, alphas)
    nc = _PROGRAM_CACHE[key]

    shared = {
        "iota": np.ascontiguousarray(iota),
        "ident": ident,
        "id16": id16,
        "w1": np.ascontiguousarray(np.asarray(inputs["W1"], dtype=np.float32)),
        "w2": w_split(inputs["W2"]), "w3": w_split(inputs["W3"]),
        "w4": w_split(inputs["W4"]), "w5": w_split(inputs["W5"]),
        "w6": w_split(inputs["W6"]),
        "b1": b_split(inputs["b1"]), "b2": b_split(inputs["b2"]),
        "b3": b_split(inputs["b3"]), "b4": b_split(inputs["b4"]),
        "b5": b_split(inputs["b5"]),
        "b6": np.ascontiguousarray(np.asarray(inputs["b6"], dtype=np.float32).reshape(P, 1)),
    }
    in_maps = []
    for c in range(N_CORES):
        b0 = c * rows_per_core
        in_maps.append({
            "hs4": np.ascontiguousarray(hs[b0:b0 + rows_per_core, :n + 1, :]),
            "cs4": np.ascontiguousarray(cs_dev[b0:b0 + rows_per_core]),
            "em4": np.ascontiguousarray(em[b0:b0 + rows_per_core]),
            **shared,
        })

    res = run_bass_kernel_spmd(nc, in_maps, list(range(N_CORES)), trace=TRACE)
    LAST_RESULT = res
    G = np.concatenate([r["g4"] for r in res.results], axis=0)  # [B, K+1, G_DIM]

    # ---- host-side Ks reassignment + G_mask (index metadata) ----
    j = np.arange(K + 1)
    small = (Ks <= K - 2)[:, None]
    move = (j[None, :] == (Ks + 1)[:, None]) & small
    G = np.where(move[..., None], G[:, K:K + 1], G)
    G = np.where(((j[None, :] == K) & small)[..., None], np.float32(0.0), G)
    G_mask = np.where((j[None, :] >= (Ks + 2)[:, None]) & small, 0.0, 1.0).astype(np.float32)
    return G.astype(np.float32), G_mask


# revision 9
# speedup vs baseline: 1.3493x; 1.3493x over previous
"""Trainium2 Bass kernel for nn_AggregateClusteredSum.

Data-parallel over the batch axis: 32 rows / 8 NeuronCores = 4 rows per core.
Per row, segment sums of hs over 64 clusters are computed as accumulating
matmuls with on-device one-hot matrices (built 8 chunks at a time by a single
DVE is_equal over zero-stride broadcast views of cs and an iota constant).
The one-hot is the stationary operand (64-wide weight loads), giving a
cluster-major [64, 128] accumulator that is transposed once per row on the
PE. The 6-layer PReLU MLP runs feature-major over all 4*129 tokens at once
(natural weight layout stationary, bias+PReLU fused into the ACT eviction),
followed by the leave-one-out aggregation on DVE and a final PE transpose
per row.

DMA structure: hs streams as four 2-MiB f32->bf16 cast transfers on the
SWDGE queue; everything else (weights, biases, iota, identities, exists
masks, cs in partition-block layout, h_n columns) is packed by the host
into ONE [128, ~3.2K] f32 tensor moved by a single HWDGE transfer, so the
queues never clog with small packets.

Host-side work is limited to index metadata (exists mask, Ks reassignment,
G_mask - pure functions of cs) plus input packing/slicing.
"""
import os
import sys

for _p in ("/opt/trn_rl_repo", "/root/.axon_site/_ro/trn_rl_repo"):
    if os.path.isdir(_p) and _p not in sys.path:
        sys.path.insert(0, _p)

import numpy as np
from contextlib import ExitStack

import concourse.bass as bass
import concourse.tile as tile
from concourse import bacc, mybir
from concourse.bass_utils import run_bass_kernel_spmd

F32 = mybir.dt.float32
F16 = mybir.dt.float16
BF16 = mybir.dt.bfloat16

N_CORES = 8
K = 64                      # clusters
H = 128                     # hidden dim of hs
G_DIM = 128                 # output dim
HID = 256                   # MLP hidden
P = 128                     # partitions
NB = 8                      # one-hot chunks built per DVE op

_PROGRAM_CACHE = {}
LAST_RESULT = None          # BassKernelResults of the most recent run (for profiling)
TRACE = False


def _pack_layout(rows_per_core, nch, rem):
    """Column offsets in the packed params tensor."""
    cs_cols = nch + (1 if rem > 0 else 0)
    off = {}
    c = 0
    off["w1"] = c; c += HID
    for li in (2, 3, 4, 5):
        for ci in range(2):
            off[f"w{li}_{ci}"] = c; c += HID
    for ci in range(2):
        off[f"w6_{ci}"] = c; c += G_DIM
    off["iota"] = c; c += K
    off["ident"] = c; c += P
    off["id16"] = c; c += K          # eye(64) in rows 0:64
    for li in range(1, 6):
        for hi in range(2):
            off[f"b{li}_{hi}"] = c; c += 1
    off["b6"] = c; c += 1
    off["hn"] = c; c += rows_per_core
    off["em"] = c; c += rows_per_core * K
    off["cs"] = c; c += rows_per_core * max(cs_cols, 1)
    off["_total"] = c
    return off


def _build_program(rows_per_core, n, alphas):
    """Build the per-core Bass program. Same program for all cores (SPMD)."""
    nch = n // P            # full 128-row chunks per batch row
    rem = n - nch * P       # remainder rows (0 for n=4096)
    ntok = 2 * K + 1        # 129 tokens per row
    T = rows_per_core * ntok  # total tokens per core (516)
    # token free-dim chunks for the MLP (PSUM bank limit: 512 f32)
    nt = (T + 511) // 512
    base = T // nt
    tchunks = []
    t0 = 0
    for i in range(nt):
        tw = base + (1 if i < T - base * nt else 0)
        tchunks.append((t0, tw))
        t0 += tw
    assert t0 == T

    cs_cols = nch + (1 if rem > 0 else 0)
    off = _pack_layout(rows_per_core, nch, rem)
    PW = off["_total"]

    nc = bacc.Bacc()
    hs_in = nc.declare_dram_parameter("hs4", [rows_per_core, n + 1, H], F32, isOutput=False)
    pk_in = nc.declare_dram_parameter("pack", [P, PW], F32, isOutput=False)
    g_out = nc.declare_dram_parameter("g4", [rows_per_core, K + 1, G_DIM], F32, isOutput=True)

    a1, a2, a3, a4, a5 = [float(a) for a in alphas]
    Act = mybir.ActivationFunctionType
    Alu = mybir.AluOpType

    with tile.TileContext(nc) as tc, ExitStack() as ctx:
        cpool = ctx.enter_context(tc.tile_pool(name="cpool", bufs=1))
        wpool = ctx.enter_context(tc.tile_pool(name="wpool", bufs=1))
        hspool = ctx.enter_context(tc.tile_pool(name="hspool", bufs=4))
        small = ctx.enter_context(tc.tile_pool(name="small", bufs=2))
        ohpool = ctx.enter_context(tc.tile_pool(name="ohpool", bufs=4))
        xpool = ctx.enter_context(tc.tile_pool(name="xpool", bufs=1))
        loopool = ctx.enter_context(tc.tile_pool(name="loopool", bufs=2))
        pseg = ctx.enter_context(tc.tile_pool(name="pseg", bufs=2, space="PSUM"))
        ptp = ctx.enter_context(tc.tile_pool(name="ptp", bufs=1, space="PSUM"))
        pmlp = ctx.enter_context(tc.tile_pool(name="pmlp", bufs=4, space="PSUM"))

        # ---- hs row loads first: SWDGE queue stays dedicated to them ----
        hs_rows = []
        for r in range(rows_per_core):
            hp = hspool.tile([P, nch * H], BF16, tag="hs", name=f"hs_{r}")
            # tile[p, c*H + h] = hs[r, p*nch + c, h]: contiguous per partition
            src = hs_in[r, 0:nch * P, :].rearrange("(p c) h -> p (c h)", p=P)
            nc.gpsimd.dma_start(out=hp[:], in_=src)
            hs_rows.append(hp)

        # ---- one packed transfer for everything else (HWDGE) ----
        pk = cpool.tile([P, PW], F32)
        nc.sync.dma_start(out=pk[:], in_=pk_in[:])

        def pslice(name, w):
            o = off[name]
            return pk[:, o:o + w]

        iota_sb = pslice("iota", K)
        ident_sb = pslice("ident", P)
        b_sb = {(li, hi): pslice(f"b{li}_{hi}", 1)
                for li in range(1, 6) for hi in range(2)}
        b6_sb = pslice("b6", 1)

        # fp16 weight tiles (converted on ACT; ACT is idle early)
        w1_sb = wpool.tile([P, HID], F16, tag="w1")
        nc.scalar.copy(w1_sb[:], pslice("w1", HID))
        w_sb = {}
        for li in (2, 3, 4, 5, 6):
            for ci in range(2):
                wdim = HID if li < 6 else G_DIM
                t = wpool.tile([P, wdim], F16, tag=f"w{li}_{ci}", name=f"w{li}_{ci}")
                nc.scalar.copy(t[:], pslice(f"w{li}_{ci}", wdim))
                w_sb[(li, ci)] = t
        id16_sb = wpool.tile([K, K], F16, tag="id16")
        nc.scalar.copy(id16_sb[:], pk[0:K, off["id16"]:off["id16"] + K])

        # X0: Hcat^T for all rows, feature-major [H, T] fp16
        x0 = xpool.tile([P, T], F16, tag="x0")

        # ---- Stage A: per-row segment sums -> X0 columns ----
        for r in range(rows_per_core):
            hp = hs_rows[r]
            cs_t = pk[:, off["cs"] + r * max(cs_cols, 1):off["cs"] + (r + 1) * max(cs_cols, 1)]
            psC = pseg.tile([K, P], F32, tag="psC", name=f"psC_{r}")
            last = (rem == 0)
            for b0 in range(0, nch, NB):
                bw = min(NB, nch - b0)
                oh = ohpool.tile([P, NB * K], BF16, tag="oh", name=f"oh_{r}_{b0}")
                cs_b = cs_t[:, b0:b0 + bw].broadcast_to((P, bw, K))
                io_b = iota_sb.unsqueeze(1).broadcast_to((P, bw, K))
                nc.vector.tensor_tensor(
                    oh[:].rearrange("p (c k) -> p c k", k=K)[:, 0:bw, :],
                    cs_b, io_b, Alu.is_equal)
                for cc in range(bw):
                    cg = b0 + cc
                    nc.tensor.matmul(
                        psC[:], oh[:, cc * K:(cc + 1) * K],
                        hp[:, cg * H:(cg + 1) * H],
                        start=(cg == 0),
                        stop=(last and cg == nch - 1))
            if rem > 0:
                hs_r = hspool.tile([P, H], BF16, tag="hs_rem")
                nc.gpsimd.dma_start(out=hs_r[0:rem, :], in_=hs_in[r, nch * P:n, :])
                oh_r = ohpool.tile([P, K], BF16, tag="oh_rem")
                nc.vector.tensor_scalar(oh_r[0:rem, :], iota_sb[0:rem, :],
                                        cs_t[0:rem, nch:nch + 1], None, Alu.is_equal)
                nc.tensor.matmul(psC[:], oh_r[0:rem, :], hs_r[0:rem, :],
                                 start=(nch == 0), stop=True)

            # cluster-major [64, 128] -> fp16 -> transpose -> [128, 64]
            cm = small.tile([K, P], F16, tag="cm", name=f"cm_{r}")
            nc.scalar.copy(cm[:], psC[:])
            tps = ptp.tile([P, K], F16, tag="tps", name=f"tps_{r}")
            nc.tensor.transpose(tps[:], cm[:], id16_sb[:])

            r0 = r * ntok
            hn_col = pk[:, off["hn"] + r:off["hn"] + r + 1]     # f32 [128,1]
            nc.scalar.copy(x0[:, r0:r0 + K], tps[:])
            nc.vector.tensor_scalar(x0[:, r0 + K:r0 + 2 * K], tps[:],
                                    hn_col, None, Alu.add)
            nc.vector.tensor_scalar(x0[:, r0 + 2 * K:r0 + 2 * K + 1],
                                    hn_col, 0.0, None, Alu.add)

        # ---- Stage B: MLP over all T tokens, feature-major ----
        x1 = [xpool.tile([P, T], F16, tag=f"x1_{h}", name=f"x1_{h}") for h in range(2)]
        for h in range(2):
            for (t0, tw) in tchunks:
                ps = pmlp.tile([P, tw], F32, tag="pmlp", name=f"ps1_{h}_{t0}")
                nc.tensor.matmul(ps[:], w1_sb[:, h * P:(h + 1) * P], x0[:, t0:t0 + tw],
                                 start=True, stop=True)
                nc.scalar.activation(x1[h][:, t0:t0 + tw], ps[:], Act.Prelu,
                                     bias=b_sb[(1, h)], scale=1.0, alpha=a1)
        xprev = x1
        for li, alpha in ((2, a2), (3, a3), (4, a4), (5, a5)):
            xn = [xpool.tile([P, T], F16, tag=f"x{li}_{h}", name=f"x{li}_{h}") for h in range(2)]
            for h in range(2):
                for (t0, tw) in tchunks:
                    ps = pmlp.tile([P, tw], F32, tag="pmlp", name=f"ps{li}_{h}_{t0}")
                    for ci in range(2):
                        nc.tensor.matmul(ps[:], w_sb[(li, ci)][:, h * P:(h + 1) * P],
                                         xprev[ci][:, t0:t0 + tw],
                                         start=(ci == 0), stop=(ci == 1))
                    nc.scalar.activation(xn[h][:, t0:t0 + tw], ps[:], Act.Prelu,
                                         bias=b_sb[(li, h)], scale=1.0, alpha=alpha)
            xprev = xn
        # L6: 256 -> 128, bias only, keep f32
        gs = xpool.tile([P, T], F32, tag="gs")
        for (t0, tw) in tchunks:
            ps = pmlp.tile([P, tw], F32, tag="pmlp", name=f"ps6_{t0}")
            for ci in range(2):
                nc.tensor.matmul(ps[:], w_sb[(6, ci)][:], xprev[ci][:, t0:t0 + tw],
                                 start=(ci == 0), stop=(ci == 1))
            nc.scalar.activation(gs[:, t0:t0 + tw], ps[:], Act.Identity,
                                 bias=b6_sb, scale=1.0)

        # ---- Stage C: leave-one-out per row; outputs packed, one store ----
        osb = loopool.tile([K + 1, rows_per_core * G_DIM], F32, tag="osb")
        for r in range(rows_per_core):
            r0 = r * ntok
            em_sb = pk[:, off["em"] + r * K:off["em"] + (r + 1) * K]
            scr = loopool.tile([P, K], F32, tag="scr", name=f"scr_{r}")
            s_col = loopool.tile([P, 1], F32, tag="scol", name=f"scol_{r}")
            # scr = gs_lo * em ; s = sum_free(scr)  (masked base sum S)
            nc.vector.scalar_tensor_tensor(scr[:], gs[:, r0:r0 + K], 1.0, em_sb,
                                           Alu.mult, Alu.mult, accum_out=s_col[:])
            gout = loopool.tile([P, K + 1], F32, tag="gout", name=f"gout_{r}")
            tmp = loopool.tile([P, K], F32, tag="tmp", name=f"tmp_{r}")
            # tmp = (gs_hi + S) - gs_lo
            nc.vector.scalar_tensor_tensor(tmp[:], gs[:, r0 + K:r0 + 2 * K], s_col[:],
                                           gs[:, r0:r0 + K], Alu.add, Alu.subtract)
            nc.vector.tensor_tensor(gout[:, 0:K], tmp[:], em_sb, Alu.mult)
            nc.vector.tensor_scalar(gout[:, K:K + 1], gs[:, r0 + 2 * K:r0 + 2 * K + 1],
                                    s_col[:], None, Alu.add)
            # transpose [128g, 65] -> [65, 128g]
            tp = ptp.tile([K + 1, P], F32, tag="tp", name=f"tp_{r}")
            nc.tensor.transpose(tp[:], gout[:], ident_sb)
            nc.scalar.copy(osb[:, r * G_DIM:(r + 1) * G_DIM], tp[:])
        nc.sync.dma_start(
            out=g_out[:].rearrange("r k g -> k r g"),
            in_=osb[:].rearrange("k (r g) -> k r g", g=G_DIM))

    nc.finalize()
    return nc


def kernel(**inputs):
    global LAST_RESULT
    hs = np.ascontiguousarray(np.asarray(inputs["hs"], dtype=np.float32))
    cs = np.asarray(inputs["cs"])
    n = int(np.asarray(inputs["n"]))
    B, N, _H = hs.shape
    assert _H == H and B % N_CORES == 0
    rows_per_core = B // N_CORES
    assert n >= 1
    nch = n // P
    npad = nch * P
    rem = n - npad
    cs_cols = nch + (1 if rem > 0 else 0)

    cs_i = cs.astype(np.int64)
    cs_valid = cs_i[:, :n]                      # entries >= n are masked off

    # ---- host-side index metadata ----
    cnt = np.zeros((B, K), dtype=np.int64)
    for b in range(B):
        cnt[b] = np.bincount(cs_valid[b], minlength=K)[:K]
    exists = (cnt > 0).astype(np.float32)       # [B, K]
    Ks = cs_valid.max(axis=1)                   # [B] (cs values are >= 0)

    # ---- packed params tensor per core ----
    off = _pack_layout(rows_per_core, nch, rem)
    PW = off["_total"]

    def wmat(x):
        return np.asarray(x, dtype=np.float32)

    packs = []
    for c in range(N_CORES):
        b0 = c * rows_per_core
        pk = np.zeros((P, PW), dtype=np.float32)
        pk[:, off["w1"]:off["w1"] + HID] = wmat(inputs["W1"])
        for li in (2, 3, 4, 5):
            w = wmat(inputs[f"W{li}"])
            for ci in range(2):
                o = off[f"w{li}_{ci}"]
                pk[:, o:o + HID] = w[ci * P:(ci + 1) * P]
        w6 = wmat(inputs["W6"])
        for ci in range(2):
            o = off[f"w6_{ci}"]
            pk[:, o:o + G_DIM] = w6[ci * P:(ci + 1) * P]
        pk[:, off["iota"]:off["iota"] + K] = np.arange(K, dtype=np.float32)[None, :]
        pk[:, off["ident"]:off["ident"] + P] = np.eye(P, dtype=np.float32)
        pk[0:K, off["id16"]:off["id16"] + K] = np.eye(K, dtype=np.float32)
        for li in range(1, 6):
            b = wmat(inputs[f"b{li}"])
            for hi in range(2):
                pk[:, off[f"b{li}_{hi}"]] = b[hi * P:(hi + 1) * P]
        pk[:, off["b6"]] = wmat(inputs["b6"])
        for r in range(rows_per_core):
            pk[:, off["hn"] + r] = hs[b0 + r, n, :]
            pk[:, off["em"] + r * K:off["em"] + (r + 1) * K] = exists[b0 + r][None, :]
            co = off["cs"] + r * max(cs_cols, 1)
            if nch > 0:
                pk[:, co:co + nch] = cs_valid[b0 + r, :npad].reshape(P, nch)
            if rem > 0:
                pk[:rem, co + nch] = cs_valid[b0 + r, npad:n]
        packs.append(pk)

    alphas = tuple(float(np.asarray(inputs[f"a{i}"])) for i in range(1, 6))
    key = (rows_per_core, n, alphas)
    if key not in _PROGRAM_CACHE:
        _PROGRAM_CACHE[key] = _build_program(rows_per_core, n, alphas)
    nc = _PROGRAM_CACHE[key]

    in_maps = []
    for c in range(N_CORES):
        b0 = c * rows_per_core
        in_maps.append({
            "hs4": np.ascontiguousarray(hs[b0:b0 + rows_per_core, :n + 1, :]),
            "pack": packs[c],
        })

    res = run_bass_kernel_spmd(nc, in_maps, list(range(N_CORES)), trace=TRACE)
    LAST_RESULT = res
    G = np.concatenate([r["g4"] for r in res.results], axis=0)  # [B, K+1, G_DIM]

    # ---- host-side Ks reassignment + G_mask (index metadata) ----
    j = np.arange(K + 1)
    small = (Ks <= K - 2)[:, None]
    move = (j[None, :] == (Ks + 1)[:, None]) & small
    G = np.where(move[..., None], G[:, K:K + 1], G)
    G = np.where(((j[None, :] == K) & small)[..., None], np.float32(0.0), G)
    G_mask = np.where((j[None, :] >= (Ks + 2)[:, None]) & small, 0.0, 1.0).astype(np.float32)
    return G.astype(np.float32), G_mask


# revision 10
# speedup vs baseline: 1.5613x; 1.1571x over previous
"""Trainium2 Bass kernel for nn_AggregateClusteredSum.

Data-parallel over the batch axis: 32 rows / 8 NeuronCores = 4 rows per core.
Per row, segment sums of hs over 64 clusters are computed as accumulating
matmuls with on-device one-hot matrices (built 8 chunks at a time by a single
DVE is_equal over zero-stride broadcast views of cs and an iota constant).
The one-hot is the stationary operand (64-wide weight loads), giving a
cluster-major [64, 128] accumulator that is transposed once per row on the
PE. The 6-layer PReLU MLP runs feature-major over all 4*129 tokens at once
(natural weight layout stationary, bias+PReLU fused into the ACT eviction),
followed by the leave-one-out aggregation on DVE and a final PE transpose
per row.

DMA structure: hs streams as four 2-MiB f32->bf16 cast transfers on the
SWDGE queue; everything else (weights, biases, iota, identities, exists
masks, cs in partition-block layout, h_n columns) is packed by the host
into ONE [128, ~3.2K] f32 tensor moved by a single HWDGE transfer, so the
queues never clog with small packets.

Host-side work is limited to index metadata (exists mask, Ks reassignment,
G_mask - pure functions of cs) plus input packing/slicing.
"""
import os
import sys

for _p in ("/opt/trn_rl_repo", "/root/.axon_site/_ro/trn_rl_repo"):
    if os.path.isdir(_p) and _p not in sys.path:
        sys.path.insert(0, _p)

import numpy as np
from contextlib import ExitStack

import concourse.bass as bass
import concourse.tile as tile
from concourse import bacc, mybir
from concourse.bass_utils import run_bass_kernel_spmd

F32 = mybir.dt.float32
F16 = mybir.dt.float16
BF16 = mybir.dt.bfloat16

N_CORES = 8
K = 64                      # clusters
H = 128                     # hidden dim of hs
G_DIM = 128                 # output dim
HID = 256                   # MLP hidden
P = 128                     # partitions
NB = 8                      # one-hot chunks built per DVE op

_PROGRAM_CACHE = {}
LAST_RESULT = None          # BassKernelResults of the most recent run (for profiling)
TRACE = False


def _pack_layout(rows_per_core, nch, rem):
    """Column offsets in the packed params tensors (f32 pack + fp16 wpack)."""
    cs_cols = nch + (1 if rem > 0 else 0)
    off = {}
    c = 0
    off["iota"] = c; c += K
    off["ident"] = c; c += P
    for li in range(1, 6):
        for hi in range(2):
            off[f"b{li}_{hi}"] = c; c += 1
    off["b6"] = c; c += 1
    off["hn"] = c; c += rows_per_core
    off["em"] = c; c += rows_per_core * K
    off["cs"] = c; c += rows_per_core * max(cs_cols, 1)
    off["_total"] = c
    # fp16 pack: weights + eye(64)
    w = {}
    c = 0
    w["w1"] = c; c += HID
    for li in (2, 3, 4, 5):
        for ci in range(2):
            w[f"w{li}_{ci}"] = c; c += HID
    for ci in range(2):
        w[f"w6_{ci}"] = c; c += G_DIM
    w["id16"] = c; c += K            # eye(64) in rows 0:64
    w["_total"] = c
    return off, w


def _build_program(rows_per_core, n, alphas):
    """Build the per-core Bass program. Same program for all cores (SPMD)."""
    nch = n // P            # full 128-row chunks per batch row
    rem = n - nch * P       # remainder rows (0 for n=4096)
    ntok = 2 * K + 1        # 129 tokens per row
    T = rows_per_core * ntok  # total tokens per core (516)
    # token free-dim chunks for the MLP (PSUM bank limit: 512 f32)
    nt = (T + 511) // 512
    base = T // nt
    tchunks = []
    t0 = 0
    for i in range(nt):
        tw = base + (1 if i < T - base * nt else 0)
        tchunks.append((t0, tw))
        t0 += tw
    assert t0 == T

    cs_cols = nch + (1 if rem > 0 else 0)
    off, woff = _pack_layout(rows_per_core, nch, rem)
    PW = off["_total"]
    WW = woff["_total"]

    nc = bacc.Bacc()
    hs_in = nc.declare_dram_parameter("hs4", [rows_per_core, n + 1, H], F32, isOutput=False)
    pk_in = nc.declare_dram_parameter("pack", [P, PW], F32, isOutput=False)
    wp_in = nc.declare_dram_parameter("wpack", [P, WW], F16, isOutput=False)
    g_out = nc.declare_dram_parameter("g4", [rows_per_core, K + 1, G_DIM], F32, isOutput=True)

    a1, a2, a3, a4, a5 = [float(a) for a in alphas]
    Act = mybir.ActivationFunctionType
    Alu = mybir.AluOpType

    with tile.TileContext(nc) as tc, ExitStack() as ctx:
        cpool = ctx.enter_context(tc.tile_pool(name="cpool", bufs=1))
        wpool = ctx.enter_context(tc.tile_pool(name="wpool", bufs=1))
        hspool = ctx.enter_context(tc.tile_pool(name="hspool", bufs=4))
        small = ctx.enter_context(tc.tile_pool(name="small", bufs=2))
        ohpool = ctx.enter_context(tc.tile_pool(name="ohpool", bufs=4))
        xpool = ctx.enter_context(tc.tile_pool(name="xpool", bufs=1))
        loopool = ctx.enter_context(tc.tile_pool(name="loopool", bufs=2))
        pseg = ctx.enter_context(tc.tile_pool(name="pseg", bufs=2, space="PSUM"))
        ptp = ctx.enter_context(tc.tile_pool(name="ptp", bufs=1, space="PSUM"))
        pmlp = ctx.enter_context(tc.tile_pool(name="pmlp", bufs=4, space="PSUM"))

        # ---- hs row loads first: SWDGE queue stays dedicated to them ----
        hs_rows = []
        for r in range(rows_per_core):
            hp = hspool.tile([P, nch * H], BF16, tag="hs", name=f"hs_{r}")
            # tile[p, c*H + h] = hs[r, p*nch + c, h]: contiguous per partition
            src = hs_in[r, 0:nch * P, :].rearrange("(p c) h -> p (c h)", p=P)
            nc.gpsimd.dma_start(out=hp[:], in_=src)
            hs_rows.append(hp)

        # ---- two packed transfers for everything else (HWDGE) ----
        pk = cpool.tile([P, PW], F32)
        nc.sync.dma_start(out=pk[:], in_=pk_in[:])
        wp = wpool.tile([P, WW], F16)
        nc.sync.dma_start(out=wp[:], in_=wp_in[:])

        def pslice(name, w):
            o = off[name]
            return pk[:, o:o + w]

        iota_sb = pslice("iota", K)
        ident_sb = pslice("ident", P)
        b_sb = {(li, hi): pslice(f"b{li}_{hi}", 1)
                for li in range(1, 6) for hi in range(2)}
        b6_sb = pslice("b6", 1)

        w1_sb = wp[:, woff["w1"]:woff["w1"] + HID]
        w_sb = {}
        for li in (2, 3, 4, 5, 6):
            for ci in range(2):
                wdim = HID if li < 6 else G_DIM
                o = woff[f"w{li}_{ci}"]
                w_sb[(li, ci)] = wp[:, o:o + wdim]
        id16_sb = wp[0:K, woff["id16"]:woff["id16"] + K]

        # X0: Hcat^T for all rows, feature-major [H, T] fp16
        x0 = xpool.tile([P, T], F16, tag="x0")

        # ---- Stage A: per-row segment sums -> X0 columns ----
        for r in range(rows_per_core):
            hp = hs_rows[r]
            cs_t = pk[:, off["cs"] + r * max(cs_cols, 1):off["cs"] + (r + 1) * max(cs_cols, 1)]
            psC = pseg.tile([K, P], F32, tag="psC", name=f"psC_{r}")
            last = (rem == 0)
            for b0 in range(0, nch, NB):
                bw = min(NB, nch - b0)
                oh = ohpool.tile([P, NB * K], BF16, tag="oh", name=f"oh_{r}_{b0}")
                cs_b = cs_t[:, b0:b0 + bw].broadcast_to((P, bw, K))
                io_b = iota_sb.unsqueeze(1).broadcast_to((P, bw, K))
                nc.vector.tensor_tensor(
                    oh[:].rearrange("p (c k) -> p c k", k=K)[:, 0:bw, :],
                    cs_b, io_b, Alu.is_equal)
                for cc in range(bw):
                    cg = b0 + cc
                    nc.tensor.matmul(
                        psC[:], oh[:, cc * K:(cc + 1) * K],
                        hp[:, cg * H:(cg + 1) * H],
                        start=(cg == 0),
                        stop=(last and cg == nch - 1))
            if rem > 0:
                hs_r = hspool.tile([P, H], BF16, tag="hs_rem")
                nc.gpsimd.dma_start(out=hs_r[0:rem, :], in_=hs_in[r, nch * P:n, :])
                oh_r = ohpool.tile([P, K], BF16, tag="oh_rem")
                nc.vector.tensor_scalar(oh_r[0:rem, :], iota_sb[0:rem, :],
                                        cs_t[0:rem, nch:nch + 1], None, Alu.is_equal)
                nc.tensor.matmul(psC[:], oh_r[0:rem, :], hs_r[0:rem, :],
                                 start=(nch == 0), stop=True)

            # cluster-major [64, 128] -> fp16 -> transpose -> [128, 64]
            cm = small.tile([K, P], F16, tag="cm", name=f"cm_{r}")
            nc.scalar.copy(cm[:], psC[:])
            tps = ptp.tile([P, K], F16, tag="tps", name=f"tps_{r}")
            nc.tensor.transpose(tps[:], cm[:], id16_sb[:])

            r0 = r * ntok
            hn_col = pk[:, off["hn"] + r:off["hn"] + r + 1]     # f32 [128,1]
            nc.scalar.copy(x0[:, r0:r0 + K], tps[:])
            nc.vector.tensor_scalar(x0[:, r0 + K:r0 + 2 * K], tps[:],
                                    hn_col, None, Alu.add)
            nc.vector.tensor_scalar(x0[:, r0 + 2 * K:r0 + 2 * K + 1],
                                    hn_col, 0.0, None, Alu.add)

        # ---- Stage B: MLP over all T tokens, feature-major ----
        x1 = [xpool.tile([P, T], F16, tag=f"x1_{h}", name=f"x1_{h}") for h in range(2)]
        for h in range(2):
            for (t0, tw) in tchunks:
                ps = pmlp.tile([P, tw], F32, tag="pmlp", name=f"ps1_{h}_{t0}")
                nc.tensor.matmul(ps[:], w1_sb[:, h * P:(h + 1) * P], x0[:, t0:t0 + tw],
                                 start=True, stop=True)
                nc.scalar.activation(x1[h][:, t0:t0 + tw], ps[:], Act.Prelu,
                                     bias=b_sb[(1, h)], scale=1.0, alpha=a1)
        xprev = x1
        for li, alpha in ((2, a2), (3, a3), (4, a4), (5, a5)):
            xn = [xpool.tile([P, T], F16, tag=f"x{li}_{h}", name=f"x{li}_{h}") for h in range(2)]
            for h in range(2):
                # ci outer, token chunk inner: one weight load serves both
                # token chunks (the PE reloads stationary per matmul)
                pss = [pmlp.tile([P, tw], F32, tag="pmlp", name=f"ps{li}_{h}_{t0}")
                       for (t0, tw) in tchunks]
                for ci in range(2):
                    for ti, (t0, tw) in enumerate(tchunks):
                        nc.tensor.matmul(pss[ti][:], w_sb[(li, ci)][:, h * P:(h + 1) * P],
                                         xprev[ci][:, t0:t0 + tw],
                                         start=(ci == 0), stop=(ci == 1))
                for ti, (t0, tw) in enumerate(tchunks):
                    nc.scalar.activation(xn[h][:, t0:t0 + tw], pss[ti][:], Act.Prelu,
                                         bias=b_sb[(li, h)], scale=1.0, alpha=alpha)
            xprev = xn
        # L6: 256 -> 128, bias only, keep f32
        gs = xpool.tile([P, T], F32, tag="gs")
        pss = [pmlp.tile([P, tw], F32, tag="pmlp", name=f"ps6_{t0}")
               for (t0, tw) in tchunks]
        for ci in range(2):
            for ti, (t0, tw) in enumerate(tchunks):
                nc.tensor.matmul(pss[ti][:], w_sb[(6, ci)][:], xprev[ci][:, t0:t0 + tw],
                                 start=(ci == 0), stop=(ci == 1))
        for ti, (t0, tw) in enumerate(tchunks):
            nc.scalar.activation(gs[:, t0:t0 + tw], pss[ti][:], Act.Identity,
                                 bias=b6_sb, scale=1.0)

        # ---- Stage C: leave-one-out per row; outputs packed, one store ----
        osb = loopool.tile([K + 1, rows_per_core * G_DIM], F32, tag="osb")
        for r in range(rows_per_core):
            r0 = r * ntok
            em_sb = pk[:, off["em"] + r * K:off["em"] + (r + 1) * K]
            scr = loopool.tile([P, K], F32, tag="scr", name=f"scr_{r}")
            s_col = loopool.tile([P, 1], F32, tag="scol", name=f"scol_{r}")
            # scr = gs_lo * em ; s = sum_free(scr)  (masked base sum S)
            nc.vector.scalar_tensor_tensor(scr[:], gs[:, r0:r0 + K], 1.0, em_sb,
                                           Alu.mult, Alu.mult, accum_out=s_col[:])
            gout = loopool.tile([P, K + 1], F32, tag="gout", name=f"gout_{r}")
            tmp = loopool.tile([P, K], F32, tag="tmp", name=f"tmp_{r}")
            # tmp = (gs_hi + S) - gs_lo
            nc.vector.scalar_tensor_tensor(tmp[:], gs[:, r0 + K:r0 + 2 * K], s_col[:],
                                           gs[:, r0:r0 + K], Alu.add, Alu.subtract)
            nc.vector.tensor_tensor(gout[:, 0:K], tmp[:], em_sb, Alu.mult)
            nc.vector.tensor_scalar(gout[:, K:K + 1], gs[:, r0 + 2 * K:r0 + 2 * K + 1],
                                    s_col[:], None, Alu.add)
            # transpose [128g, 65] -> [65, 128g]
            tp = ptp.tile([K + 1, P], F32, tag="tp", name=f"tp_{r}")
            nc.tensor.transpose(tp[:], gout[:], ident_sb)
            nc.scalar.copy(osb[:, r * G_DIM:(r + 1) * G_DIM], tp[:])
        nc.sync.dma_start(
            out=g_out[:].rearrange("r k g -> k r g"),
            in_=osb[:].rearrange("k (r g) -> k r g", g=G_DIM))

    nc.finalize()
    return nc


def kernel(**inputs):
    global LAST_RESULT
    hs = np.ascontiguousarray(np.asarray(inputs["hs"], dtype=np.float32))
    cs = np.asarray(inputs["cs"])
    n = int(np.asarray(inputs["n"]))
    B, N, _H = hs.shape
    assert _H == H and B % N_CORES == 0
    rows_per_core = B // N_CORES
    assert n >= 1
    nch = n // P
    npad = nch * P
    rem = n - npad
    cs_cols = nch + (1 if rem > 0 else 0)

    cs_i = cs.astype(np.int64)
    cs_valid = cs_i[:, :n]                      # entries >= n are masked off

    # ---- host-side index metadata ----
    cnt = np.zeros((B, K), dtype=np.int64)
    for b in range(B):
        cnt[b] = np.bincount(cs_valid[b], minlength=K)[:K]
    exists = (cnt > 0).astype(np.float32)       # [B, K]
    Ks = cs_valid.max(axis=1)                   # [B] (cs values are >= 0)

    # ---- packed params tensors per core ----
    off, woff = _pack_layout(rows_per_core, nch, rem)
    PW = off["_total"]
    WW = woff["_total"]

    def wmat(x):
        return np.asarray(x, dtype=np.float32)

    wpack = np.zeros((P, WW), dtype=np.float16)
    wpack[:, woff["w1"]:woff["w1"] + HID] = wmat(inputs["W1"]).astype(np.float16)
    for li in (2, 3, 4, 5):
        w = wmat(inputs[f"W{li}"]).astype(np.float16)
        for ci in range(2):
            o = woff[f"w{li}_{ci}"]
            wpack[:, o:o + HID] = w[ci * P:(ci + 1) * P]
    w6 = wmat(inputs["W6"]).astype(np.float16)
    for ci in range(2):
        o = woff[f"w6_{ci}"]
        wpack[:, o:o + G_DIM] = w6[ci * P:(ci + 1) * P]
    wpack[0:K, woff["id16"]:woff["id16"] + K] = np.eye(K, dtype=np.float16)

    packs = []
    for c in range(N_CORES):
        b0 = c * rows_per_core
        pk = np.zeros((P, PW), dtype=np.float32)
        pk[:, off["iota"]:off["iota"] + K] = np.arange(K, dtype=np.float32)[None, :]
        pk[:, off["ident"]:off["ident"] + P] = np.eye(P, dtype=np.float32)
        for li in range(1, 6):
            b = wmat(inputs[f"b{li}"])
            for hi in range(2):
                pk[:, off[f"b{li}_{hi}"]] = b[hi * P:(hi + 1) * P]
        pk[:, off["b6"]] = wmat(inputs["b6"])
        for r in range(rows_per_core):
            pk[:, off["hn"] + r] = hs[b0 + r, n, :]
            pk[:, off["em"] + r * K:off["em"] + (r + 1) * K] = exists[b0 + r][None, :]
            co = off["cs"] + r * max(cs_cols, 1)
            if nch > 0:
                pk[:, co:co + nch] = cs_valid[b0 + r, :npad].reshape(P, nch)
            if rem > 0:
                pk[:rem, co + nch] = cs_valid[b0 + r, npad:n]
        packs.append(pk)

    alphas = tuple(float(np.asarray(inputs[f"a{i}"])) for i in range(1, 6))
    key = (rows_per_core, n, alphas)
    if key not in _PROGRAM_CACHE:
        _PROGRAM_CACHE[key] = _build_program(rows_per_core, n, alphas)
    nc = _PROGRAM_CACHE[key]

    in_maps = []
    for c in range(N_CORES):
        b0 = c * rows_per_core
        in_maps.append({
            "hs4": np.ascontiguousarray(hs[b0:b0 + rows_per_core, :n + 1, :]),
            "pack": packs[c],
            "wpack": wpack,
        })

    res = run_bass_kernel_spmd(nc, in_maps, list(range(N_CORES)), trace=TRACE)
    LAST_RESULT = res
    G = np.concatenate([r["g4"] for r in res.results], axis=0)  # [B, K+1, G_DIM]

    # ---- host-side Ks reassignment + G_mask (index metadata) ----
    j = np.arange(K + 1)
    small = (Ks <= K - 2)[:, None]
    move = (j[None, :] == (Ks + 1)[:, None]) & small
    G = np.where(move[..., None], G[:, K:K + 1], G)
    G = np.where(((j[None, :] == K) & small)[..., None], np.float32(0.0), G)
    G_mask = np.where((j[None, :] >= (Ks + 2)[:, None]) & small, 0.0, 1.0).astype(np.float32)
    return G.astype(np.float32), G_mask
